# revision 1
# baseline (speedup 1.0000x reference)
"""KoLeo loss kernel for Trainium2, 8 NeuronCores.

Strategy (data-parallel brute-force 1-NN over L2-normalized rows):
  - Each core gets a row-PERMUTED copy of x with its own 1024 rows first, so
    the self-match diagonal always falls in columns 0..1023 (core-invariant
    program, as required by SPMD).
  - On device: normalize rows in f32, cast to fp16, DMA-roundtrip through DRAM
    with XBAR transpose to build xT [128, 6, 8192] (D on partitions).
  - dots slab = xT_own_cols^T @ xT (fp16 matmul, f32 PSUM accumulate).
    Diagonal masked by adding -2*I at the known chunk/offset; running
    elementwise max over 512-col chunks; final row-max m.
  - pdist for normalized vectors: dist = sqrt(2 - 2*m)  (the reference's +EPS
    inside the diff perturbs the scalar loss by ~1e-8 relative - negligible).
  - loss partial per core = sum(log(dist + EPS)); host combines:
    loss = -(sum partials) / 8192.
"""

import sys

sys.path.insert(0, "/opt/trn_rl_repo")

import numpy as np

import concourse.bass as bass
import concourse.mybir as mybir
import concourse.tile as tile
from concourse import bacc
from concourse.bass_utils import run_bass_kernel_spmd

B = 8192
D = 768
NCORES = 8
RPC = B // NCORES  # 1024 rows per core
P = 128
KC = D // P  # 6 contraction chunks
CH = 512  # moving chunk width
NCH = B // CH  # 16 chunks
T = RPC // P  # 8 row tiles per core
EPS = 1e-8

f32 = mybir.dt.float32
f16 = mybir.dt.float16
AF = mybir.ActivationFunctionType
ALU = mybir.AluOpType
AX = mybir.AxisListType


def _build_program():
    nc = bacc.Bacc("TRN2", target_bir_lowering=False, debug=False, enable_asserts=True)
    x_in = nc.dram_tensor("xm", [B, D], f32, kind="ExternalInput").ap()
    mask4_in = nc.dram_tensor("mask4", [P, 4, CH], f32, kind="ExternalInput").ap()
    ones_in = nc.dram_tensor("ones", [P, 1], f32, kind="ExternalInput").ap()
    consts_in = nc.dram_tensor("consts", [P, 2], f32, kind="ExternalInput").ap()
    out_t = nc.dram_tensor("partial", [1, 1], f32, kind="ExternalOutput").ap()

    with tile.TileContext(nc) as tc:
        with (
            tc.tile_pool(name="big", bufs=1) as big,
            tc.tile_pool(name="work", bufs=3) as work,
            tc.tile_pool(name="psum", bufs=4, space="PSUM") as psum_pool,
            tc.tile_pool(name="dram", bufs=1, space="DRAM") as dram_pool,
        ):
            # persistent tiles
            xTn = [big.tile([P, KC, CH], f16, name=f"xT{n}", tag=f"xT{n}") for n in range(NCH)]
            mask4 = big.tile([P, 4, CH], f32, tag="mask4")
            ones = big.tile([P, 1], f32, tag="ones")
            consts = big.tile([P, 2], f32, tag="consts")
            accs = [big.tile([P, CH], f32, name=f"acc{t}", tag=f"acc{t}") for t in range(T)]

            nc.sync.dma_start(mask4[:], mask4_in)
            nc.sync.dma_start(ones[:], ones_in)
            nc.sync.dma_start(consts[:], consts_in)
            two = consts[:, 0:1]
            epsb = consts[:, 1:2]

            xn_dram = dram_pool.tile([B, D], f16, name="xn_dram")

            # Phase A: normalize rows -> fp16 -> DRAM; per 512-row chunk,
            # 6 transposing DMAs back into xTn[chunk].
            for j in range(B // P):  # 64 row tiles
                rt = work.tile([P, D], f32, tag="rt")
                nc.sync.dma_start(rt[:], x_in[j * P : (j + 1) * P, :])
                sq = work.tile([P, D], f32, tag="sq")
                ss = work.tile([P, 1], f32, tag="ss")
                nc.scalar.activation(sq[:], rt[:], AF.Square, accum_out=ss[:])
                nrm = work.tile([P, 1], f32, tag="nrm")
                nc.scalar.activation(nrm[:], ss[:], AF.Sqrt)
                nrmc = work.tile([P, 1], f32, tag="nrmc")
                nc.vector.tensor_scalar_max(nrmc[:], nrm[:], float(EPS))
                rinv = work.tile([P, 1], f32, tag="rinv")
                nc.vector.reciprocal(rinv[:], nrmc[:])
                xn = work.tile([P, D], f16, tag="xn")
                nc.scalar.mul(xn[:], rt[:], rinv[:])
                nc.sync.dma_start(xn_dram[j * P : (j + 1) * P, :], xn[:])
                if j % 4 == 3:
                    n = j // 4
                    for c in range(KC):
                        nc.sync.dma_start_transpose(
                            xTn[n][:, c, :],
                            xn_dram[n * CH : (n + 1) * CH, c * P : (c + 1) * P],
                        )

            # Phase B: matmul + running max
            for n in range(NCH):
                for t in range(T):
                    pt = psum_pool.tile([P, CH], f32, tag="pt")
                    for c in range(KC):
                        nt = t // 4  # chunk holding this row tile's columns
                        nc.tensor.matmul(
                            pt[:],
                            lhsT=xTn[nt][:, c, (t % 4) * P : (t % 4 + 1) * P],
                            rhs=xTn[n][:, c, :],
                            start=(c == 0),
                            stop=(c == KC - 1),
                        )
                    if n == t // 4:
                        v = t % 4
                        if n == 0:
                            nc.vector.tensor_tensor(
                                out=accs[t][:], in0=pt[:], in1=mask4[:, v],
                                op=ALU.add,
                            )
                        else:
                            tmp = work.tile([P, CH], f32, tag="tmp")
                            nc.vector.tensor_tensor(
                                out=tmp[:], in0=pt[:], in1=mask4[:, v], op=ALU.add
                            )
                            nc.vector.tensor_tensor(
                                out=accs[t][:], in0=accs[t][:], in1=tmp[:],
                                op=ALU.max,
                            )
                    elif n == 0:
                        nc.vector.tensor_copy(out=accs[t][:], in_=pt[:])
                    else:
                        nc.vector.tensor_tensor(
                            out=accs[t][:], in0=accs[t][:], in1=pt[:], op=ALU.max
                        )

            # Phase C: row max -> dist -> log -> partial sum
            rmax = big.tile([P, T], f32, tag="rmax")
            for t in range(T):
                nc.vector.tensor_reduce(
                    rmax[:, t : t + 1], accs[t][:], axis=AX.X, op=ALU.max
                )
            dist = big.tile([P, T], f32, tag="dist")
            nc.scalar.activation(dist[:], rmax[:], AF.Sqrt, scale=-2.0, bias=two)
            logd = big.tile([P, T], f32, tag="logd")
            lsum = big.tile([P, 1], f32, tag="lsum")
            nc.scalar.activation(
                logd[:], dist[:], AF.Ln, bias=epsb, accum_out=lsum[:]
            )
            pfin = psum_pool.tile([1, 1], f32, tag="pfin")
            nc.tensor.matmul(pfin[:], lhsT=ones[:], rhs=lsum[:], start=True, stop=True)
            res = big.tile([1, 1], f32, tag="res")
            nc.vector.tensor_copy(out=res[:], in_=pfin[:])
            nc.sync.dma_start(out_t[:], res[:])

    nc.compile()
    return nc


_NC_CACHE = None


def _get_nc():
    global _NC_CACHE
    if _NC_CACHE is None:
        _NC_CACHE = _build_program()
    return _NC_CACHE


def _make_in_maps(x: np.ndarray):
    mask4 = np.zeros((P, 4, CH), dtype=np.float32)
    for v in range(4):
        mask4[:, v, v * P : (v + 1) * P] = -2.0 * np.eye(P, dtype=np.float32)
    ones = np.ones((P, 1), dtype=np.float32)
    consts = np.zeros((P, 2), dtype=np.float32)
    consts[:, 0] = 2.0
    consts[:, 1] = EPS
    in_maps = []
    for m in range(NCORES):
        own = x[m * RPC : (m + 1) * RPC]
        rest = np.concatenate([x[: m * RPC], x[(m + 1) * RPC :]], axis=0)
        xm = np.ascontiguousarray(np.concatenate([own, rest], axis=0))
        in_maps.append({"xm": xm, "mask4": mask4, "ones": ones, "consts": consts})
    return in_maps


def kernel(student_output: np.ndarray) -> np.ndarray:
    x = np.asarray(student_output, dtype=np.float32)
    nc = _get_nc()
    in_maps = _make_in_maps(x)
    res = run_bass_kernel_spmd(nc, in_maps, list(range(NCORES)))
    total = 0.0
    for r in res.results:
        total += float(r["partial"].reshape(()))
    loss = -(total / B)
    return np.float32(loss)



# revision 24
# speedup vs baseline: 1.2633x; 1.2633x over previous
"""KoLeo loss kernel for Trainium2, 8 NeuronCores.

Strategy (data-parallel brute-force 1-NN over L2-normalized rows):
  - Each core gets a row-PERMUTED copy of x with its own 1024 rows first, so
    the self-match diagonal always falls in columns 0..1023 (core-invariant
    program, as required by SPMD).
  - On device: normalize rows in f32, cast to fp16, DMA-roundtrip through DRAM
    with XBAR transpose to build xT [128, 6, 8192] (D on partitions).
  - dots slab = xT_own_cols^T @ xT (fp16 matmul, f32 PSUM accumulate).
    Each [128,512] PSUM chunk is row-max-reduced directly into a per-chunk
    slot (diag chunks get a -2*I mask add first); a final 16-way reduce per
    row tile yields the row max m.
  - PE warm-up: zero-valued fp16 matmuls accumulate +0 into the first real
    PSUM group, so the PE's HAM clock gate reaches 8/8 before the real
    matmuls begin (and the warm-up is not DCE-able).
  - pdist for normalized vectors: dist = sqrt(2 - 2*m); per-core partial
    loss = sum(log(dist + EPS)); host combines: loss = -(sum partials)/8192.
"""

import os
import sys

sys.path.insert(0, "/opt/trn_rl_repo")

import numpy as np

import concourse.bass as bass
import concourse.mybir as mybir
import concourse.tile as tile
from concourse import bacc
from concourse.bass_utils import run_bass_kernel_spmd

B = 8192
D = 768
NCORES = 8
RPC = B // NCORES  # 1024 rows per core
P = 128
KC = D // P  # 6 contraction chunks
CH = 512  # moving chunk width
NCH = B // CH  # 16 chunks
T = RPC // P  # 8 row tiles per core
EPS = 1e-8
NWARM = int(os.environ.get("K_NWARM", "48"))  # HAM warm-up matmuls
K_SQ = os.environ.get("K_SQ", "0") == "1"  # transposes also on scalar queue

f32 = mybir.dt.float32
f16 = mybir.dt.float16
AF = mybir.ActivationFunctionType
ALU = mybir.AluOpType
AX = mybir.AxisListType


def _build_program():
    nc = bacc.Bacc("TRN2", target_bir_lowering=False, debug=False,
                   enable_asserts=True)
    x_in = nc.dram_tensor("xm", [B, D], f32, kind="ExternalInput").ap()
    # planes 0-3: -2*I diag masks at offsets 0..3; plane 4: zeros
    mask4_in = nc.dram_tensor("mask4", [P, 5, CH], f32, kind="ExternalInput").ap()
    consts_in = nc.dram_tensor("consts", [P, 3], f32, kind="ExternalInput").ap()
    wz_in = nc.dram_tensor("wz", [P, CH], f16, kind="ExternalInput").ap()
    out_t = nc.dram_tensor("partial", [1, 1], f32, kind="ExternalOutput").ap()

    with tile.TileContext(nc) as tc:
        with (
            tc.tile_pool(name="big", bufs=1) as big,
            tc.tile_pool(name="work", bufs=3) as work,
            tc.tile_pool(name="small", bufs=4) as small,
            tc.tile_pool(name="psum", bufs=4, space="PSUM") as psum_pool,
            tc.tile_pool(name="dram", bufs=1, space="DRAM") as dram_pool,
        ):
            # persistent tiles
            xTn = [big.tile([P, KC, CH], f16, name=f"xT{n}", tag=f"xT{n}")
                   for n in range(NCH)]
            mask4 = big.tile([P, 5, CH], f32, tag="mask4")
            consts = big.tile([P, 3], f32, tag="consts")
            wz = big.tile([P, CH], f16, tag="wz")
            rmax0 = big.tile([P, T], f32, tag="rmax0")
            rmaxall = big.tile([P, NCH, T], f32, tag="rmaxall")

            nc.sync.dma_start(mask4[:], mask4_in)
            nc.sync.dma_start(consts[:], consts_in)
            nc.sync.dma_start(wz[:], wz_in)
            ones = consts[:, 0:1]
            two = consts[:, 1:2]
            epsb = consts[:, 2:3]
            warm_l = wz[:, 0:P]

            xn_dram = dram_pool.tile([B, D], f16, name="xn_dram")

            # Phase A: normalize rows -> fp16 -> DRAM; per 512-row chunk,
            # 6 transposing DMAs back into xTn[chunk].
            for j in range(B // P):  # 64 row tiles
                rt = work.tile([P, D], f32, tag="rt")
                nc.sync.dma_start(rt[:], x_in[j * P : (j + 1) * P, :])
                sq = work.tile([P, D], f32, tag="sq")
                ss = small.tile([P, 1], f32, tag="ss")
                nc.scalar.activation(sq[:], rt[:], AF.Square, accum_out=ss[:])
                nrm = small.tile([P, 1], f32, tag="nrm")
                nc.scalar.activation(nrm[:], ss[:], AF.Sqrt)
                rinv = small.tile([P, 1], f32, tag="rinv")
                nc.vector.reciprocal(rinv[:], nrm[:])
                xn = work.tile([P, D], f16, tag="xn")
                nc.scalar.mul(xn[:], rt[:], rinv[:])
                nc.sync.dma_start(xn_dram[j * P : (j + 1) * P, :], xn[:])
                if j % 4 == 3:
                    n = j // 4
                    for c in range(KC):
                        eng = nc.scalar if (K_SQ and c % 2 == 1) else nc.sync
                        eng.dma_start_transpose(
                            xTn[n][:, c, :],
                            xn_dram[n * CH : (n + 1) * CH, c * P : (c + 1) * P],
                        )

            # Phase B: matmul + per-chunk row max into slots
            for n in range(NCH):
                for t in range(T):
                    pt = psum_pool.tile([P, CH], f32, tag="pt")
                    if n == 0 and t == 0:
                        # HAM warm-up: accumulate +0 into the first group
                        for w in range(NWARM):
                            nc.tensor.matmul(
                                pt[:], lhsT=warm_l, rhs=wz[:],
                                start=(w == 0), stop=False,
                            )
                    for c in range(KC):
                        nt = t // 4  # chunk holding this row tile's columns
                        nc.tensor.matmul(
                            pt[:],
                            lhsT=xTn[nt][:, c, (t % 4) * P : (t % 4 + 1) * P],
                            rhs=xTn[n][:, c, :],
                            start=(c == 0 and not (n == 0 and t == 0)),
                            stop=(c == KC - 1),
                        )
                    slot = rmaxall[:, n, t : t + 1]
                    if n == t // 4:
                        scr = work.tile([P, CH], f32, tag="dscr")
                        nc.vector.tensor_tensor(
                            out=scr[:], in0=pt[:], in1=mask4[:, t % 4],
                            op=ALU.add,
                        )
                        nc.vector.tensor_reduce(slot, scr[:], axis=AX.X,
                                                op=ALU.max)
                    else:
                        nc.vector.tensor_reduce(slot, pt[:], axis=AX.X,
                                                op=ALU.max)

            # Phase C: merge slots -> dist -> log -> partial sum
            for t in range(T):
                nc.vector.tensor_reduce(
                    rmax0[:, t : t + 1], rmaxall[:, :, t], axis=AX.X,
                    op=ALU.max,
                )
            dist = big.tile([P, T], f32, tag="dist")
            nc.scalar.activation(dist[:], rmax0[:], AF.Sqrt, scale=-2.0,
                                 bias=two)
            logd = big.tile([P, T], f32, tag="logd")
            lsum = big.tile([P, 1], f32, tag="lsum")
            nc.scalar.activation(
                logd[:], dist[:], AF.Ln, bias=epsb, accum_out=lsum[:]
            )
            pfin = psum_pool.tile([1, 1], f32, tag="pfin")
            nc.tensor.matmul(pfin[:], lhsT=ones, rhs=lsum[:], start=True,
                             stop=True)
            res = big.tile([1, 1], f32, tag="res")
            nc.vector.tensor_copy(out=res[:], in_=pfin[:])
            nc.sync.dma_start(out_t[:], res[:])

    nc.compile()
    return nc


_NC_CACHE = None


def _get_nc():
    global _NC_CACHE
    if _NC_CACHE is None:
        _NC_CACHE = _build_program()
    return _NC_CACHE


def _make_in_maps(x: np.ndarray):
    mask4 = np.zeros((P, 5, CH), dtype=np.float32)
    for v in range(4):
        mask4[:, v, v * P : (v + 1) * P] = -2.0 * np.eye(P, dtype=np.float32)
    consts = np.zeros((P, 3), dtype=np.float32)
    consts[:, 0] = 1.0
    consts[:, 1] = 2.0
    consts[:, 2] = EPS
    wz = np.zeros((P, CH), dtype=np.float16)
    in_maps = []
    for m in range(NCORES):
        own = x[m * RPC : (m + 1) * RPC]
        rest = np.concatenate([x[: m * RPC], x[(m + 1) * RPC :]], axis=0)
        xm = np.ascontiguousarray(np.concatenate([own, rest], axis=0))
        in_maps.append({"xm": xm, "mask4": mask4, "consts": consts, "wz": wz})
    return in_maps


def kernel(student_output: np.ndarray) -> np.ndarray:
    x = np.asarray(student_output, dtype=np.float32)
    nc = _get_nc()
    in_maps = _make_in_maps(x)
    res = run_bass_kernel_spmd(nc, in_maps, list(range(NCORES)))
    total = 0.0
    for r in res.results:
        total += float(r["partial"].reshape(()))
    loss = -(total / B)
    return np.float32(loss)
